# revision 1
# baseline (speedup 1.0000x reference)
"""GATv2 2-layer GNN message-passing kernel for Trainium2, 8-core SPMD.

Contract: kernel(**inputs) takes the FULL unsharded inputs (as produced by
setup_inputs) and returns the FULL [50000, 128] float32 output.

Strategy (edge/data parallel, dst-range sharded):
- Host: append self-loops, sort edges by dst, give each of the 8 cores an
  equal contiguous dst-node range (6250 nodes = 49 blocks of 128). Within
  each block, edges are split by src-half so the int16 dma_gather indices
  stay < 32768 (two source tables). Per-block group counts are padded to a
  uniform (max) count so one SPMD program serves all cores.
- Device, per block of 128 dst nodes: batched dma_gather of xl[src] (lo+hi
  tables) and xr_local[dst]; z = xl+xr (DVE); LeakyReLU (ACT Prelu);
  scores = per-head reduce of att*lrelu(z); w = exp(scores) (softmax
  shift-invariance lets us skip the segment max -- scores are O(10));
  u = w*z; selection matrix S[e,j] = (dst_rel[e]==j) built via is_equal
  against an iota row; PE matmuls accumulate S^T @ [u | w] into the block
  PSUM, giving both sum_e w*z*[dst==j] and the softmax denominators.
  Epilogue: out = relu((psum_feat - xr*denom) / (denom+1e-16) + bias),
  using sum w*z = sum w*xl + xr*denom to recover sum w*xl exactly.
- Between layers: each core computes xl2 = h1_local @ W2_l for its slab,
  AllGather replicates the xl2 table; xr2 stays local (only local dst
  needed). Layer-2 gather indices address the rank-slab layout.
"""
import sys
sys.path.insert(0, '/opt/trn_rl_repo')
import numpy as np
from dataclasses import dataclass

import concourse.bass as bass
import concourse.bacc as bacc
import concourse.mybir as mybir
from concourse.tile import TileContext
from concourse.library_config import mlp
from concourse.masks import make_identity
from concourse.bass_utils import run_bass_kernel_spmd

P = 128
H, C = 4, 32
D = H * C          # 128
SLOPE = 0.2
F32 = mybir.dt.float32
I16 = mybir.dt.int16


@dataclass
class Plan:
    N: int
    NC: int
    NPC: int        # nodes per core
    NBLK: int       # blocks per core
    SLAB: int       # NBLK*128
    G_lo: int
    G_hi: int
    split_rank: int

    @property
    def GPB(self):
        return self.G_lo + self.G_hi


def wrap_idx(flat):
    """[n] int -> dma_gather SBUF layout [128, n//16] (16-wrapped, 8x replicated)."""
    n = flat.shape[0]
    assert n % 16 == 0
    w = flat.reshape(n // 16, 16).T      # [16, n/16]
    return np.tile(w, (8, 1)).astype(np.int16)


def preprocess(x, edge_index, NC=8):
    """Build the per-core streams. Returns (plan, per_core_dict_list)."""
    N = x.shape[0]
    assert N % NC == 0
    NPC = N // NC
    NBLK = (NPC + P - 1) // P
    SLAB = NBLK * P
    split_rank = NC // 2
    SPLIT1 = split_rank * NPC          # layer-1 lo/hi split (global node id)
    assert SPLIT1 <= 32768 and N - SPLIT1 <= 32768
    assert split_rank * SLAB <= 32768 and (NC - split_rank) * SLAB <= 32768

    loop = np.arange(N, dtype=np.int64)
    src = np.concatenate([np.asarray(edge_index[0]), loop]).astype(np.int64)
    dst = np.concatenate([np.asarray(edge_index[1]), loop]).astype(np.int64)

    order = np.argsort(dst, kind='stable')
    src = src[order].astype(np.int32)
    dst = dst[order].astype(np.int32)

    core_bounds = np.searchsorted(dst, np.arange(NC + 1) * NPC)

    per_core = []
    G_lo = G_hi = 1
    for k in range(NC):
        a, b = core_bounds[k], core_bounds[k + 1]
        s_k = src[a:b]
        d_k = dst[a:b] - k * NPC
        blk = d_k // P
        is_lo = s_k < SPLIT1
        lo_counts = np.bincount(blk[is_lo], minlength=NBLK)
        hi_counts = np.bincount(blk[~is_lo], minlength=NBLK)
        G_lo = max(G_lo, int(np.max((lo_counts + P - 1) // P)) or 1)
        G_hi = max(G_hi, int(np.max((hi_counts + P - 1) // P)) or 1)
        per_core.append((s_k, d_k, blk, is_lo))

    plan = Plan(N=N, NC=NC, NPC=NPC, NBLK=NBLK, SLAB=SLAB,
                G_lo=G_lo, G_hi=G_hi, split_rank=split_rank)
    GPB = plan.GPB

    datas = []
    for k in range(NC):
        s_k, d_k, blk, is_lo = per_core[k]
        idxA1 = np.zeros((NBLK, GPB * P), np.int16)
        idxA2 = np.zeros((NBLK, GPB * P), np.int16)
        idxB = np.zeros((NBLK, GPB * P), np.int16)
        dstrel = np.full((NBLK, GPB * P), -1.0, np.float32)
        for b in range(NBLK):
            in_b = blk == b
            for side, G0, Gn in ((True, 0, G_lo), (False, G_lo, G_hi)):
                sel = in_b & (is_lo == side)
                ss = s_k[sel]
                dd = d_k[sel]
                n = ss.shape[0]
                assert n <= Gn * P
                o = G0 * P
                if side:
                    idxA1[b, o:o + n] = ss
                    idxA2[b, o:o + n] = (ss // NPC) * SLAB + (ss % NPC)
                else:
                    idxA1[b, o:o + n] = ss - SPLIT1
                    idxA2[b, o:o + n] = ((ss // NPC) * SLAB + (ss % NPC)
                                         - split_rank * SLAB)
                idxB[b, o:o + n] = dd
                dstrel[b, o:o + n] = dd - b * P

        def wrap_blocks(arr):
            return np.stack([wrap_idx(arr[b]) for b in range(NBLK)])

        wA1 = wrap_blocks(idxA1)
        wA2 = wrap_blocks(idxA2)
        wB = wrap_blocks(idxB)
        blkidx_l1 = np.concatenate([wA1, wB], axis=2).reshape(NBLK * P, 2 * GPB * 8)
        blkidx_l2 = np.concatenate([wA2, wB], axis=2).reshape(NBLK * P, 2 * GPB * 8)
        dr = dstrel.reshape(NBLK, GPB, P).transpose(0, 2, 1).reshape(NBLK * P, GPB)
        datas.append(dict(blkidx_l1=blkidx_l1, blkidx_l2=blkidx_l2,
                          dstrel=np.ascontiguousarray(dr)))
    return plan, datas


def build_kernel(plan, lrelu_on_act=True, repeat=1):
    """Build the SPMD nc program (identical for all cores)."""
    pl = plan
    GPB, G_lo, G_hi, NBLK, SLAB = pl.GPB, pl.G_lo, pl.G_hi, pl.NBLK, pl.SLAB
    NLO1 = pl.split_rank * pl.NPC
    NLO2 = pl.split_rank * SLAB

    nc = bacc.Bacc("TRN2", target_bir_lowering=False, debug=False)
    dp = lambda name, shape, dt=F32, out=False: nc.declare_dram_parameter(
        name, list(shape), dt, isOutput=out).ap()

    xl1 = dp("xl1", [pl.N, D])
    xr1_loc = dp("xr1_loc", [SLAB, D])
    blkidx_l1 = dp("blkidx_l1", [NBLK * P, 2 * GPB * 8], I16)
    blkidx_l2 = dp("blkidx_l2", [NBLK * P, 2 * GPB * 8], I16)
    dstrel_p = dp("dstrel", [NBLK * P, GPB])
    att1_t = dp("att1_t", [P, D])
    att2_t = dp("att2_t", [P, D])
    iota_p = dp("iota", [P, P])
    W2l_p = dp("W2l", [D, D])
    W2r_p = dp("W2r", [D, D])
    bias1_p = dp("bias1", [P, D])
    bias2_p = dp("bias2", [P, D])
    out_p = dp("out", [SLAB, D], out=True)

    h1_loc = nc.dram_tensor("h1_loc", [SLAB, D], F32).ap()
    xl2_slab = nc.dram_tensor("xl2_slab", [SLAB, D], F32).ap()
    xl2_full = nc.dram_tensor("xl2_full", [pl.NC * SLAB, D], F32,
                              addr_space="Shared").ap()
    xr2_loc = nc.dram_tensor("xr2_loc", [SLAB, D], F32).ap()

    with TileContext(nc) as tc:
        nc.gpsimd.load_library(mlp)
        with (
            tc.tile_pool(name="const", bufs=1) as cpool,
            tc.tile_pool(name="stream", bufs=3) as spool,
            tc.tile_pool(name="work", bufs=2) as wpool,
            tc.tile_pool(name="small", bufs=3) as smpool,
            tc.tile_pool(name="psum", bufs=2, space="PSUM") as pspool,
            tc.tile_pool(name="psum2", bufs=2, space="PSUM") as ps2pool,
        ):
            att1_c = cpool.tile([P, D], F32)
            nc.sync.dma_start(out=att1_c[:], in_=att1_t[:, :])
            att2_c = cpool.tile([P, D], F32)
            nc.sync.dma_start(out=att2_c[:], in_=att2_t[:, :])
            iota_c = cpool.tile([P, P], F32)
            nc.sync.dma_start(out=iota_c[:], in_=iota_p[:, :])
            W2l_c = cpool.tile([D, D], F32)
            nc.sync.dma_start(out=W2l_c[:], in_=W2l_p[:, :])
            W2r_c = cpool.tile([D, D], F32)
            nc.sync.dma_start(out=W2r_c[:], in_=W2r_p[:, :])
            bias1_c = cpool.tile([P, D], F32)
            nc.sync.dma_start(out=bias1_c[:], in_=bias1_p[:, :])
            bias2_c = cpool.tile([P, D], F32)
            nc.sync.dma_start(out=bias2_c[:], in_=bias2_p[:, :])
            ident_c = cpool.tile([P, P], F32)
            make_identity(nc, ident_c[:])
            alpha_c = cpool.tile([P, 1], F32)
            nc.vector.memset(alpha_c[:], SLOPE)

            def lrelu(out_ap, in_ap):
                if lrelu_on_act:
                    nc.scalar.activation(out=out_ap, in_=in_ap,
                                         func=mybir.ActivationFunctionType.Prelu,
                                         alpha=alpha_c[:, :])
                else:
                    nc.vector.scalar_tensor_tensor(
                        out=out_ap, in0=in_ap, scalar=SLOPE, in1=in_ap,
                        op0=mybir.AluOpType.mult, op1=mybir.AluOpType.max)

            GS = max(G_lo, G_hi)

            def edge_layer(tab_lo, tab_hi, tab_B, blkidx, att_c, bias_c,
                           out_rows, xr_loc_ap):
                sides = [(0, 0, G_lo, tab_lo), (1, G_lo, G_hi, tab_hi)]
                sides = [s for s in sides if s[2] > 0]
                for b in range(NBLK):
                    idx_t = spool.tile([P, 2 * GPB * 8], I16, tag="idx")
                    nc.sync.dma_start(out=idx_t[:],
                                      in_=blkidx[b * P:(b + 1) * P, :])
                    dr_t = spool.tile([P, GPB], F32, tag="dr")
                    nc.sync.dma_start(out=dr_t[:],
                                      in_=dstrel_p[b * P:(b + 1) * P, :])

                    ps = pspool.tile([P, D + H], F32, tag="agg")

                    for si, (side, G0, Gn, tab) in enumerate(sides):
                        sl = slice(0, Gn)
                        za = wpool.tile([P, GS, D], F32, tag="za")
                        zb = wpool.tile([P, GS, D], F32, tag="zb")
                        GCH = 8  # ring limit: <=1024 idx (64 descs/lane) per call
                        for g0 in range(0, Gn, GCH):
                            gn = min(GCH, Gn - g0)
                            nc.gpsimd.dma_gather(
                                out_ap=za[:, g0:g0 + gn, :], in_ap=tab,
                                idxs_ap=idx_t[:, (G0 + g0) * 8:(G0 + g0 + gn) * 8],
                                num_idxs=gn * P, num_idxs_reg=gn * P, elem_size=D)
                            nc.gpsimd.dma_gather(
                                out_ap=zb[:, g0:g0 + gn, :], in_ap=tab_B,
                                idxs_ap=idx_t[:, (GPB + G0 + g0) * 8:
                                              (GPB + G0 + g0 + gn) * 8],
                                num_idxs=gn * P, num_idxs_reg=gn * P, elem_size=D)
                        z = za  # reuse za as z
                        nc.vector.tensor_tensor(out=z[:, sl, :], in0=za[:, sl, :],
                                                in1=zb[:, sl, :],
                                                op=mybir.AluOpType.add)
                        lz = wpool.tile([P, GS, D], F32, tag="lz")
                        lrelu(lz[:, sl, :], z[:, sl, :])
                        m = zb  # reuse zb as m
                        nc.vector.tensor_tensor(
                            out=m[:, sl, :], in0=lz[:, sl, :],
                            in1=att_c[:].unsqueeze(1).to_broadcast([P, Gn, D]),
                            op=mybir.AluOpType.mult)
                        e_t = smpool.tile([P, GS, H], F32, tag="e")
                        nc.vector.tensor_reduce(
                            out=e_t[:, sl, :],
                            in_=m[:, sl, :].rearrange("p g (h c) -> p g h c", h=H),
                            axis=mybir.AxisListType.X, op=mybir.AluOpType.add)
                        w_t = smpool.tile([P, GS, H], F32, tag="w")
                        nc.scalar.activation(out=w_t[:, sl, :], in_=e_t[:, sl, :],
                                             func=mybir.ActivationFunctionType.Exp)
                        rhs = wpool.tile([P, GS, D + H], F32, tag="rhs")
                        nc.vector.tensor_tensor(
                            out=rhs[:, sl, 0:D].rearrange("p g (h c) -> p g h c", h=H),
                            in0=z[:, sl, :].rearrange("p g (h c) -> p g h c", h=H),
                            in1=w_t[:, sl, :].unsqueeze(3).to_broadcast([P, Gn, H, C]),
                            op=mybir.AluOpType.mult)
                        nc.vector.tensor_copy(out=rhs[:, sl, D:D + H],
                                              in_=w_t[:, sl, :])
                        S_t = wpool.tile([P, GS, P], F32, tag="S")
                        nc.vector.tensor_tensor(
                            out=S_t[:, sl, :],
                            in0=iota_c[:].unsqueeze(1).to_broadcast([P, Gn, P]),
                            in1=dr_t[:, G0:G0 + Gn].unsqueeze(2).to_broadcast(
                                [P, Gn, P]),
                            op=mybir.AluOpType.is_equal)
                        for gi in range(Gn):
                            nc.tensor.matmul(
                                out=ps[:], lhsT=S_t[:, gi, :], rhs=rhs[:, gi, :],
                                start=(si == 0 and gi == 0),
                                stop=(si == len(sides) - 1 and gi == Gn - 1))

                    xrb = smpool.tile([P, D], F32, tag="xrb")
                    nc.sync.dma_start(out=xrb[:],
                                      in_=xr_loc_ap[b * P:(b + 1) * P, :])
                    deps = smpool.tile([P, H], F32, tag="deps")
                    nc.vector.tensor_scalar_add(out=deps[:], in0=ps[:, D:D + H],
                                                scalar1=1e-16)
                    dinv = smpool.tile([P, H], F32, tag="dinv")
                    nc.vector.reciprocal(out=dinv[:], in_=deps[:])
                    t1 = smpool.tile([P, D], F32, tag="t1")
                    nc.vector.tensor_tensor(
                        out=t1[:].rearrange("p (h c) -> p h c", h=H),
                        in0=xrb[:].rearrange("p (h c) -> p h c", h=H),
                        in1=ps[:, D:D + H].unsqueeze(2).to_broadcast([P, H, C]),
                        op=mybir.AluOpType.mult)
                    t2 = smpool.tile([P, D], F32, tag="t2")
                    nc.vector.tensor_tensor(out=t2[:], in0=ps[:, 0:D], in1=t1[:],
                                            op=mybir.AluOpType.subtract)
                    t3 = smpool.tile([P, D], F32, tag="t3")
                    nc.vector.tensor_tensor(
                        out=t3[:].rearrange("p (h c) -> p h c", h=H),
                        in0=t2[:].rearrange("p (h c) -> p h c", h=H),
                        in1=dinv[:].unsqueeze(2).to_broadcast([P, H, C]),
                        op=mybir.AluOpType.mult)
                    t4 = smpool.tile([P, D], F32, tag="t4")
                    nc.vector.tensor_tensor(out=t4[:], in0=t3[:], in1=bias_c[:],
                                            op=mybir.AluOpType.add)
                    hrow = smpool.tile([P, D], F32, tag="hrow")
                    nc.scalar.activation(out=hrow[:], in_=t4[:],
                                         func=mybir.ActivationFunctionType.Relu)
                    nc.sync.dma_start(out=out_rows[b * P:(b + 1) * P, :],
                                      in_=hrow[:])

            for _rep in range(repeat):
                edge_layer(xl1[0:NLO1, :], xl1[NLO1:pl.N, :], xr1_loc[:, :],
                           blkidx_l1, att1_c, bias1_c, h1_loc, xr1_loc)

                for b in range(NBLK):
                    htile = smpool.tile([P, D], F32, tag="pl_h")
                    nc.sync.dma_start(out=htile[:],
                                      in_=h1_loc[b * P:(b + 1) * P, :])
                    psT = ps2pool.tile([P, P], F32, tag="pl_T")
                    nc.tensor.transpose(out=psT[:], in_=htile[:],
                                        identity=ident_c[:])
                    hT = smpool.tile([P, P], F32, tag="pl_hT")
                    nc.vector.tensor_copy(out=hT[:], in_=psT[:])
                    for W_c, table in ((W2l_c, xl2_slab), (W2r_c, xr2_loc)):
                        psm = ps2pool.tile([P, D], F32, tag="pl_mm")
                        nc.tensor.matmul(out=psm[:], lhsT=hT[:], rhs=W_c[:],
                                         start=True, stop=True)
                        res = smpool.tile([P, D], F32, tag="pl_res")
                        nc.vector.tensor_copy(out=res[:], in_=psm[:])
                        nc.sync.dma_start(out=table[b * P:(b + 1) * P, :],
                                          in_=res[:])

                nc.gpsimd.collective_compute(
                    "AllGather", mybir.AluOpType.bypass,
                    replica_groups=[list(range(pl.NC))],
                    ins=[xl2_slab[:, :].opt()],
                    outs=[xl2_full[:, :].opt()],
                )

                edge_layer(xl2_full[0:NLO2, :], xl2_full[NLO2:pl.NC * SLAB, :],
                           xr2_loc[:, :], blkidx_l2, att2_c, bias2_c, out_p,
                           xr2_loc)

    return nc


def make_inputs(plan, datas, x, W1_l, W1_r, att1, b1, W2_l, W2_r, att2, b2):
    pl = plan
    xl1 = (x @ W1_l).astype(np.float32)
    xr1 = (x @ W1_r).astype(np.float32)
    att1_t = np.tile(np.asarray(att1).reshape(1, D), (P, 1)).astype(np.float32)
    att2_t = np.tile(np.asarray(att2).reshape(1, D), (P, 1)).astype(np.float32)
    iota = np.tile(np.arange(P, dtype=np.float32)[None, :], (P, 1))
    bias1_t = np.tile(np.asarray(b1).reshape(1, D), (P, 1)).astype(np.float32)
    bias2_t = np.tile(np.asarray(b2).reshape(1, D), (P, 1)).astype(np.float32)

    in_maps = []
    for k in range(pl.NC):
        xr1_loc = np.zeros((pl.SLAB, D), np.float32)
        nreal = min(pl.NPC, pl.N - k * pl.NPC)
        xr1_loc[:nreal] = xr1[k * pl.NPC: k * pl.NPC + nreal]
        in_maps.append(dict(
            xl1=xl1,
            xr1_loc=xr1_loc,
            blkidx_l1=datas[k]["blkidx_l1"],
            blkidx_l2=datas[k]["blkidx_l2"],
            dstrel=datas[k]["dstrel"],
            att1_t=att1_t, att2_t=att2_t, iota=iota,
            W2l=np.asarray(W2_l, np.float32), W2r=np.asarray(W2_r, np.float32),
            bias1=bias1_t, bias2=bias2_t,
        ))
    return in_maps


def assemble_output(plan, results):
    out = np.zeros((plan.N, D), np.float32)
    for k in range(plan.NC):
        out[k * plan.NPC:(k + 1) * plan.NPC] = results[k]["out"][:plan.NPC]
    return out


def kernel(x, edge_index, W1_l, W1_r, att1, b1, W2_l, W2_r, att2, b2):
    x = np.ascontiguousarray(np.asarray(x, np.float32))
    edge_index = np.asarray(edge_index)
    plan, datas = preprocess(x, edge_index, NC=8)
    nc = build_kernel(plan, lrelu_on_act=True)
    nc.compile()
    in_maps = make_inputs(plan, datas, x, np.asarray(W1_l), np.asarray(W1_r),
                          att1, b1, np.asarray(W2_l), np.asarray(W2_r),
                          att2, b2)
    res = run_bass_kernel_spmd(nc, in_maps, core_ids=list(range(8)))
    return assemble_output(plan, res.results)



# revision 2
# speedup vs baseline: 2.5604x; 2.5604x over previous
"""GATv2 2-layer GNN message-passing kernel for Trainium2, 8-core SPMD.

Contract: kernel(**inputs) takes the FULL unsharded inputs (as produced by
setup_inputs) and returns the FULL [50000, 128] float32 output.

Strategy (edge/data parallel, dst-range sharded), v2:
- Host: append self-loops, sort edges by dst, give each of the 8 cores an
  equal contiguous dst-node range (6250 nodes = 49 blocks of 128). Within
  each block, edges are split by src-half so the int16 dma_gather indices
  stay < 32768 (two source tables). Per-block group counts are padded to a
  uniform (max) count so one SPMD program serves all cores.
- All edge-path tensors are bf16. The gather tables are pre-scaled by
  |att| with columns permuted so each head's positive-att columns come
  first: the per-edge attention dot then reduces to
  e = sum_pos Prelu(z~) - sum_neg Prelu(z~), killing the att multiply.
  The sign flip is 4 ragged 4x-mode tensor_scalar negations; the sum is a
  log2 tree of 2x-mode adds (tensor_reduce runs at 1x and is avoided).
- w broadcast (rhs = w (x) z~) and the scatter one-hot build (S = iota ==
  dstrel) would run at 1x due to stride-0 innermost broadcast; both use a
  pair-duplicate + int32-view copy chain so every wide op runs at 2x.
- Per block of 128 dst nodes: PE matmuls accumulate S^T @ [w*z~ | w] into
  PSUM, giving sum_e w*z~*[dst==j] and the softmax denominators.
  Epilogue: out = relu((psum_feat - xr~*denom) / denom + bias') where
  bias' = s*b; the |att| scale is left folded in h1 and undone via the
  host-transformed W2 matrices (relu commutes with positive scales), and
  the final output is unscaled/unpermuted on the host.
- Between layers: each core computes xl2~ = h1 @ W2l' for its slab
  (W2l' absorbs layer-1 unscale + layer-2 scale/permutation), AllGather
  (bf16) replicates the xl2~ table; xr2~ stays local.
"""
import sys
sys.path.insert(0, '/opt/trn_rl_repo')
import numpy as np
from dataclasses import dataclass

import concourse.bass as bass
import concourse.bacc as bacc
import concourse.mybir as mybir
from concourse.tile import TileContext
from concourse.library_config import mlp
from concourse.bass_utils import run_bass_kernel_spmd

P = 128
H, C = 4, 32
D = H * C          # 128
SLOPE = 0.2
F32 = mybir.dt.float32
BF16 = mybir.dt.bfloat16
I16 = mybir.dt.int16
I32 = mybir.dt.int32
NPBF = mybir.dt.np(BF16)


@dataclass
class Plan:
    N: int
    NC: int
    NPC: int        # nodes per core
    NBLK: int       # blocks per core
    SLAB: int       # NBLK*128
    G_lo: int
    G_hi: int
    split_rank: int

    @property
    def GPB(self):
        return self.G_lo + self.G_hi


def wrap_idx(flat):
    """[n] int -> dma_gather SBUF layout [128, n//16] (16-wrapped, 8x replicated)."""
    n = flat.shape[0]
    assert n % 16 == 0
    w = flat.reshape(n // 16, 16).T      # [16, n/16]
    return np.tile(w, (8, 1)).astype(np.int16)


def preprocess(x, edge_index, NC=8):
    """Build the per-core streams. Returns (plan, per_core_dict_list)."""
    N = x.shape[0]
    assert N % NC == 0
    NPC = N // NC
    NBLK = (NPC + P - 1) // P
    SLAB = NBLK * P
    split_rank = NC // 2
    SPLIT1 = split_rank * NPC          # layer-1 lo/hi split (global node id)
    assert SPLIT1 <= 32768 and N - SPLIT1 <= 32768
    assert split_rank * SLAB <= 32768 and (NC - split_rank) * SLAB <= 32768

    loop = np.arange(N, dtype=np.int64)
    src = np.concatenate([np.asarray(edge_index[0]), loop]).astype(np.int64)
    dst = np.concatenate([np.asarray(edge_index[1]), loop]).astype(np.int64)

    order = np.argsort(dst, kind='stable')
    src = src[order].astype(np.int32)
    dst = dst[order].astype(np.int32)

    core_bounds = np.searchsorted(dst, np.arange(NC + 1) * NPC)

    per_core = []
    G_lo = G_hi = 1
    for k in range(NC):
        a, b = core_bounds[k], core_bounds[k + 1]
        s_k = src[a:b]
        d_k = dst[a:b] - k * NPC
        blk = d_k // P
        is_lo = s_k < SPLIT1
        lo_counts = np.bincount(blk[is_lo], minlength=NBLK)
        hi_counts = np.bincount(blk[~is_lo], minlength=NBLK)
        G_lo = max(G_lo, int(np.max((lo_counts + P - 1) // P)) or 1)
        G_hi = max(G_hi, int(np.max((hi_counts + P - 1) // P)) or 1)
        per_core.append((s_k, d_k, blk, is_lo))

    plan = Plan(N=N, NC=NC, NPC=NPC, NBLK=NBLK, SLAB=SLAB,
                G_lo=G_lo, G_hi=G_hi, split_rank=split_rank)
    GPB = plan.GPB

    datas = []
    for k in range(NC):
        s_k, d_k, blk, is_lo = per_core[k]
        idxA1 = np.zeros((NBLK, GPB * P), np.int16)
        idxA2 = np.zeros((NBLK, GPB * P), np.int16)
        idxB = np.zeros((NBLK, GPB * P), np.int16)
        dstrel = np.full((NBLK, GPB * P), -1.0, np.float32)
        for b in range(NBLK):
            in_b = blk == b
            for side, G0, Gn in ((True, 0, G_lo), (False, G_lo, G_hi)):
                sel = in_b & (is_lo == side)
                ss = s_k[sel]
                dd = d_k[sel]
                n = ss.shape[0]
                assert n <= Gn * P
                o = G0 * P
                if side:
                    idxA1[b, o:o + n] = ss
                    idxA2[b, o:o + n] = (ss // NPC) * SLAB + (ss % NPC)
                else:
                    idxA1[b, o:o + n] = ss - SPLIT1
                    idxA2[b, o:o + n] = ((ss // NPC) * SLAB + (ss % NPC)
                                         - split_rank * SLAB)
                idxB[b, o:o + n] = dd
                dstrel[b, o:o + n] = dd - b * P

        def wrap_blocks(arr):
            return np.stack([wrap_idx(arr[b]) for b in range(NBLK)])

        wA1 = wrap_blocks(idxA1)
        wA2 = wrap_blocks(idxA2)
        wB = wrap_blocks(idxB)
        blkidx_l1 = np.concatenate([wA1, wB], axis=2).reshape(NBLK * P, 2 * GPB * 8)
        blkidx_l2 = np.concatenate([wA2, wB], axis=2).reshape(NBLK * P, 2 * GPB * 8)
        dr = dstrel.reshape(NBLK, GPB, P).transpose(0, 2, 1).reshape(NBLK * P, GPB)
        datas.append(dict(blkidx_l1=blkidx_l1, blkidx_l2=blkidx_l2,
                          dstrel=np.ascontiguousarray(dr).astype(NPBF)))
    return plan, datas


def sign_perm(att):
    """Permutation putting each head's positive-att columns first.

    Returns (perm[128], scales s=|att|[perm], pos-counts per head)."""
    a = np.asarray(att, np.float32).reshape(H, C)
    perm = []
    pcounts = []
    for h in range(H):
        pos = np.nonzero(a[h] >= 0)[0]
        neg = np.nonzero(a[h] < 0)[0]
        perm.extend((h * C + pos).tolist() + (h * C + neg).tolist())
        pcounts.append(len(pos))
    perm = np.asarray(perm, np.int64)
    flat = np.abs(a.reshape(-1))[perm]
    return perm, flat.astype(np.float32), pcounts


def build_kernel(plan, pcounts1, pcounts2, repeat=1):
    """Build the SPMD nc program (identical for all cores)."""
    pl = plan
    GPB, G_lo, G_hi, NBLK, SLAB = pl.GPB, pl.G_lo, pl.G_hi, pl.NBLK, pl.SLAB
    NLO1 = pl.split_rank * pl.NPC
    NLO2 = pl.split_rank * SLAB
    A = mybir.AluOpType

    nc = bacc.Bacc("TRN2", target_bir_lowering=False, debug=False)
    dp = lambda name, shape, dt=BF16, out=False: nc.declare_dram_parameter(
        name, list(shape), dt, isOutput=out).ap()

    xl1 = dp("xl1", [pl.N, D])
    xr1_loc = dp("xr1_loc", [SLAB, D])
    blkidx_l1 = dp("blkidx_l1", [NBLK * P, 2 * GPB * 8], I16)
    blkidx_l2 = dp("blkidx_l2", [NBLK * P, 2 * GPB * 8], I16)
    dstrel_p = dp("dstrel", [NBLK * P, GPB])
    iota_p = dp("iota", [P, P])
    ident_p = dp("ident", [P, P])
    W2l_p = dp("W2l", [D, D])
    W2r_p = dp("W2r", [D, D])
    bias1_p = dp("bias1", [P, D], F32)
    bias2_p = dp("bias2", [P, D], F32)
    out_p = dp("out", [SLAB, D], out=True)

    h1_loc = nc.dram_tensor("h1_loc", [SLAB, D], BF16).ap()
    xl2_slab = nc.dram_tensor("xl2_slab", [SLAB, D], BF16).ap()
    xl2_full = nc.dram_tensor("xl2_full", [pl.NC * SLAB, D], BF16,
                              addr_space="Shared").ap()
    xr2_loc = nc.dram_tensor("xr2_loc", [SLAB, D], BF16).ap()

    with TileContext(nc) as tc:
        nc.gpsimd.load_library(mlp)
        with (
            tc.tile_pool(name="const", bufs=1) as cpool,
            tc.tile_pool(name="stream", bufs=3) as spool,
            tc.tile_pool(name="work", bufs=2) as wpool,
            tc.tile_pool(name="small", bufs=3) as smpool,
            tc.tile_pool(name="psum", bufs=2, space="PSUM") as pspool,
            tc.tile_pool(name="psum2", bufs=2, space="PSUM") as ps2pool,
        ):
            iota_c = cpool.tile([P, P], BF16)
            nc.sync.dma_start(out=iota_c[:], in_=iota_p[:, :])
            ident_c = cpool.tile([P, P], BF16)
            nc.sync.dma_start(out=ident_c[:], in_=ident_p[:, :])
            W2l_c = cpool.tile([D, D], BF16)
            nc.sync.dma_start(out=W2l_c[:], in_=W2l_p[:, :])
            W2r_c = cpool.tile([D, D], BF16)
            nc.sync.dma_start(out=W2r_c[:], in_=W2r_p[:, :])
            bias1_c = cpool.tile([P, D], F32)
            nc.sync.dma_start(out=bias1_c[:], in_=bias1_p[:, :])
            bias2_c = cpool.tile([P, D], F32)
            nc.sync.dma_start(out=bias2_c[:], in_=bias2_p[:, :])
            alpha_c = cpool.tile([P, 1], F32)
            nc.vector.memset(alpha_c[:], SLOPE)

            def edge_layer(tab_lo, tab_hi, tab_B, blkidx, pcounts, bias_c,
                           out_rows, xr_loc_ap):
                sides = [(0, G_lo, tab_lo), (G_lo, G_hi, tab_hi)]
                sides = [s for s in sides if s[1] > 0]
                for b in range(NBLK):
                    idx_t = spool.tile([P, 2 * GPB * 8], I16, tag="idx")
                    nc.sync.dma_start(out=idx_t[:],
                                      in_=blkidx[b * P:(b + 1) * P, :])
                    dr_t = spool.tile([P, GPB], BF16, tag="dr")
                    nc.sync.dma_start(out=dr_t[:],
                                      in_=dstrel_p[b * P:(b + 1) * P, :])

                    za = wpool.tile([P, GPB, D], BF16, tag="za")
                    zb = wpool.tile([P, GPB, D], BF16, tag="zb")
                    GCH = 8  # ring limit: <=1024 idx (64 descs/lane) per call
                    for G0, Gn, tab in sides:
                        for g0 in range(0, Gn, GCH):
                            gn = min(GCH, Gn - g0)
                            nc.gpsimd.dma_gather(
                                out_ap=za[:, G0 + g0:G0 + g0 + gn, :], in_ap=tab,
                                idxs_ap=idx_t[:, (G0 + g0) * 8:(G0 + g0 + gn) * 8],
                                num_idxs=gn * P, num_idxs_reg=gn * P, elem_size=D)
                            nc.gpsimd.dma_gather(
                                out_ap=zb[:, G0 + g0:G0 + g0 + gn, :], in_ap=tab_B,
                                idxs_ap=idx_t[:, (GPB + G0 + g0) * 8:
                                              (GPB + G0 + g0 + gn) * 8],
                                num_idxs=gn * P, num_idxs_reg=gn * P, elem_size=D)

                    # z~ = xl~ + xr~ (in-place into za); Prelu -> zb
                    nc.vector.tensor_tensor(out=za[:], in0=za[:], in1=zb[:],
                                            op=A.add)
                    nc.scalar.activation(out=zb[:], in_=za[:],
                                         func=mybir.ActivationFunctionType.Prelu,
                                         alpha=alpha_c[:, :])
                    # sign fix: negate each head's negative-att column block
                    zbh = zb[:].rearrange("p g (h c) -> p g h c", h=H)
                    for h in range(H):
                        ph = pcounts[h]
                        if ph < C:
                            nc.vector.tensor_scalar_mul(
                                out=zbh[:, :, h, ph:C], in0=zbh[:, :, h, ph:C],
                                scalar1=-1.0)
                    # tree-reduce over c (2x-mode adds; last step to fp32)
                    e16 = smpool.tile([P, GPB, H, 16], BF16, tag="e16")
                    nc.vector.tensor_tensor(out=e16[:], in0=zbh[:, :, :, 0:16],
                                            in1=zbh[:, :, :, 16:32], op=A.add)
                    e8 = smpool.tile([P, GPB, H, 8], BF16, tag="e8")
                    nc.vector.tensor_tensor(out=e8[:], in0=e16[:, :, :, 0:8],
                                            in1=e16[:, :, :, 8:16], op=A.add)
                    e4 = smpool.tile([P, GPB, H, 4], BF16, tag="e4")
                    nc.vector.tensor_tensor(out=e4[:], in0=e8[:, :, :, 0:4],
                                            in1=e8[:, :, :, 4:8], op=A.add)
                    e2 = smpool.tile([P, GPB, H, 2], BF16, tag="e2")
                    nc.vector.tensor_tensor(out=e2[:], in0=e4[:, :, :, 0:2],
                                            in1=e4[:, :, :, 2:4], op=A.add)
                    e1 = smpool.tile([P, GPB, H], F32, tag="e1")
                    nc.vector.tensor_tensor(out=e1[:], in0=e2[:, :, :, 0],
                                            in1=e2[:, :, :, 1], op=A.add)
                    # w = exp(e)
                    w_t = smpool.tile([P, GPB, H], BF16, tag="w")
                    nc.scalar.activation(out=w_t[:], in_=e1[:],
                                         func=mybir.ActivationFunctionType.Exp)
                    # expand w to [P,G,H,C] via pair-dup + int32-view copies
                    wdup = smpool.tile([P, GPB, H, 2], BF16, tag="wdup")
                    nc.vector.tensor_copy(
                        out=wdup[:],
                        in_=w_t[:].unsqueeze(3).to_broadcast([P, GPB, H, 2]))
                    wfull = wpool.tile([P, GPB, D], BF16, tag="wfull")
                    nc.vector.tensor_copy(
                        out=wfull[:].bitcast(I32).rearrange(
                            "p g (h c) -> p g h c", h=H),
                        in_=wdup[:].bitcast(I32).unsqueeze(3).to_broadcast(
                            [P, GPB, H, C // 2]))
                    # rhs = [w * z~ | w]
                    rhs = wpool.tile([P, GPB, D + H], BF16, tag="rhs")
                    nc.vector.tensor_tensor(out=rhs[:, :, 0:D], in0=za[:],
                                            in1=wfull[:], op=A.mult)
                    nc.vector.tensor_copy(out=rhs[:, :, D:D + H], in_=w_t[:])
                    # S = (iota == dstrel) via pair-dup + int32-view expand
                    drdup = smpool.tile([P, GPB, 2], BF16, tag="drdup")
                    nc.vector.tensor_copy(
                        out=drdup[:],
                        in_=dr_t[:].unsqueeze(2).to_broadcast([P, GPB, 2]))
                    drfull = wpool.tile([P, GPB, P], BF16, tag="drfull")
                    nc.vector.tensor_copy(
                        out=drfull[:].bitcast(I32),
                        in_=drdup[:].bitcast(I32).unsqueeze(2).to_broadcast(
                            [P, GPB, P // 2]))
                    S_t = wpool.tile([P, GPB, P], BF16, tag="S")
                    nc.vector.tensor_tensor(
                        out=S_t[:],
                        in0=iota_c[:].unsqueeze(1).to_broadcast([P, GPB, P]),
                        in1=drfull[:], op=A.is_equal)

                    ps = pspool.tile([P, D + H], F32, tag="agg")
                    for gi in range(GPB):
                        nc.tensor.matmul(
                            out=ps[:], lhsT=S_t[:, gi, :], rhs=rhs[:, gi, :],
                            start=(gi == 0), stop=(gi == GPB - 1))

                    xrb = smpool.tile([P, D], BF16, tag="xrb")
                    nc.sync.dma_start(out=xrb[:],
                                      in_=xr_loc_ap[b * P:(b + 1) * P, :])
                    deps = smpool.tile([P, H], F32, tag="deps")
                    nc.vector.tensor_scalar_add(out=deps[:], in0=ps[:, D:D + H],
                                                scalar1=1e-16)
                    dinv = smpool.tile([P, H], F32, tag="dinv")
                    nc.vector.reciprocal(out=dinv[:], in_=deps[:])
                    t1 = smpool.tile([P, D], F32, tag="t1")
                    nc.vector.tensor_tensor(
                        out=t1[:].rearrange("p (h c) -> p h c", h=H),
                        in0=xrb[:].rearrange("p (h c) -> p h c", h=H),
                        in1=ps[:, D:D + H].unsqueeze(2).to_broadcast([P, H, C]),
                        op=A.mult)
                    t2 = smpool.tile([P, D], F32, tag="t2")
                    nc.vector.tensor_tensor(out=t2[:], in0=ps[:, 0:D], in1=t1[:],
                                            op=A.subtract)
                    t3 = smpool.tile([P, D], F32, tag="t3")
                    nc.vector.tensor_tensor(
                        out=t3[:].rearrange("p (h c) -> p h c", h=H),
                        in0=t2[:].rearrange("p (h c) -> p h c", h=H),
                        in1=dinv[:].unsqueeze(2).to_broadcast([P, H, C]),
                        op=A.mult)
                    t4 = smpool.tile([P, D], F32, tag="t4")
                    nc.vector.tensor_tensor(out=t4[:], in0=t3[:], in1=bias_c[:],
                                            op=A.add)
                    hrow = smpool.tile([P, D], BF16, tag="hrow")
                    nc.vector.tensor_scalar_max(out=hrow[:], in0=t4[:],
                                                scalar1=0.0)
                    nc.sync.dma_start(out=out_rows[b * P:(b + 1) * P, :],
                                      in_=hrow[:])

            for _rep in range(repeat):
                edge_layer(xl1[0:NLO1, :], xl1[NLO1:pl.N, :], xr1_loc[:, :],
                           blkidx_l1, pcounts1, bias1_c, h1_loc, xr1_loc)

                for b in range(NBLK):
                    htile = smpool.tile([P, D], BF16, tag="pl_h")
                    nc.sync.dma_start(out=htile[:],
                                      in_=h1_loc[b * P:(b + 1) * P, :])
                    psT = ps2pool.tile([P, P], F32, tag="pl_T")
                    nc.tensor.transpose(out=psT[:], in_=htile[:],
                                        identity=ident_c[:])
                    hT = smpool.tile([P, P], BF16, tag="pl_hT")
                    nc.vector.tensor_copy(out=hT[:], in_=psT[:])
                    for W_c, table in ((W2l_c, xl2_slab), (W2r_c, xr2_loc)):
                        psm = ps2pool.tile([P, D], F32, tag="pl_mm")
                        nc.tensor.matmul(out=psm[:], lhsT=hT[:], rhs=W_c[:],
                                         start=True, stop=True)
                        res = smpool.tile([P, D], BF16, tag="pl_res")
                        nc.vector.tensor_copy(out=res[:], in_=psm[:])
                        nc.sync.dma_start(out=table[b * P:(b + 1) * P, :],
                                          in_=res[:])

                nc.gpsimd.collective_compute(
                    "AllGather", A.bypass,
                    replica_groups=[list(range(pl.NC))],
                    ins=[xl2_slab[:, :].opt()],
                    outs=[xl2_full[:, :].opt()],
                )

                edge_layer(xl2_full[0:NLO2, :], xl2_full[NLO2:pl.NC * SLAB, :],
                           xr2_loc[:, :], blkidx_l2, pcounts2, bias2_c, out_p,
                           xr2_loc)

    return nc


def make_host_tables(x, W1_l, W1_r, att1, b1, W2_l, W2_r, att2, b2):
    """Host-side scaled/permuted tables and constants."""
    perm1, s1, pc1 = sign_perm(att1)
    perm2, s2, pc2 = sign_perm(att2)
    x = np.asarray(x, np.float32)
    xl1 = (x @ np.asarray(W1_l, np.float32))[:, perm1] * s1
    xr1 = (x @ np.asarray(W1_r, np.float32))[:, perm1] * s1
    # W2': rows in perm1 order, unscaled by 1/s1; cols in perm2 order, scaled
    W2l = (np.asarray(W2_l, np.float32)[perm1][:, perm2] * s2) / s1[:, None]
    W2r = (np.asarray(W2_r, np.float32)[perm1][:, perm2] * s2) / s1[:, None]
    bias1 = np.asarray(b1, np.float32)[perm1] * s1
    bias2 = np.asarray(b2, np.float32)[perm2] * s2
    return dict(perm1=perm1, s1=s1, pc1=pc1, perm2=perm2, s2=s2, pc2=pc2,
                xl1=xl1.astype(NPBF), xr1=xr1.astype(NPBF),
                W2l=W2l.astype(NPBF), W2r=W2r.astype(NPBF),
                bias1=np.tile(bias1[None, :], (P, 1)).astype(np.float32),
                bias2=np.tile(bias2[None, :], (P, 1)).astype(np.float32))


def make_inputs(plan, datas, ht):
    pl = plan
    iota = np.tile(np.arange(P, dtype=np.float32)[None, :], (P, 1)).astype(NPBF)
    ident = np.eye(P, dtype=np.float32).astype(NPBF)

    in_maps = []
    for k in range(pl.NC):
        xr1_loc = np.zeros((pl.SLAB, D), NPBF)
        nreal = min(pl.NPC, pl.N - k * pl.NPC)
        xr1_loc[:nreal] = ht['xr1'][k * pl.NPC: k * pl.NPC + nreal]
        in_maps.append(dict(
            xl1=ht['xl1'],
            xr1_loc=xr1_loc,
            blkidx_l1=datas[k]["blkidx_l1"],
            blkidx_l2=datas[k]["blkidx_l2"],
            dstrel=datas[k]["dstrel"],
            iota=iota, ident=ident,
            W2l=ht['W2l'], W2r=ht['W2r'],
            bias1=ht['bias1'], bias2=ht['bias2'],
        ))
    return in_maps


def assemble_output(plan, results, ht):
    out = np.zeros((plan.N, D), np.float32)
    for k in range(plan.NC):
        out[k * plan.NPC:(k + 1) * plan.NPC] = \
            results[k]["out"][:plan.NPC].astype(np.float32)
    # undo layer-2 column scale+permutation
    full = np.empty_like(out)
    full[:, ht['perm2']] = out / ht['s2'][None, :]
    return full


def kernel(x, edge_index, W1_l, W1_r, att1, b1, W2_l, W2_r, att2, b2):
    x = np.ascontiguousarray(np.asarray(x, np.float32))
    edge_index = np.asarray(edge_index)
    plan, datas = preprocess(x, edge_index, NC=8)
    ht = make_host_tables(x, W1_l, W1_r, att1, b1, W2_l, W2_r, att2, b2)
    nc = build_kernel(plan, ht['pc1'], ht['pc2'])
    nc.compile()
    in_maps = make_inputs(plan, datas, ht)
    res = run_bass_kernel_spmd(nc, in_maps, core_ids=list(range(8)))
    return assemble_output(plan, res.results)


# revision 9
# speedup vs baseline: 7.4103x; 2.8942x over previous
"""GATv2 2-layer GNN message-passing kernel for Trainium2, 8-core SPMD.

Contract: kernel(**inputs) takes the FULL unsharded inputs (as produced by
setup_inputs) and returns the FULL [50000, 128] float32 output.

Strategy (edge/data parallel, dst-range sharded), v2:
- Host: append self-loops, sort edges by dst, give each of the 8 cores an
  equal contiguous dst-node range (6250 nodes = 49 blocks of 128). Within
  each block, edges are split by src-half so the int16 dma_gather indices
  stay < 32768 (two source tables). Per-block group counts are padded to a
  uniform (max) count so one SPMD program serves all cores.
- All edge-path tensors are bf16. The gather tables are pre-scaled by
  |att| with columns permuted so each head's positive-att columns come
  first: the per-edge attention dot then reduces to
  e = sum_pos Prelu(z~) - sum_neg Prelu(z~), killing the att multiply.
  The sign flip is 4 ragged 4x-mode tensor_scalar negations; the sum is a
  log2 tree of 2x-mode adds (tensor_reduce runs at 1x and is avoided).
- w broadcast (rhs = w (x) z~) and the scatter one-hot build (S = iota ==
  dstrel) would run at 1x due to stride-0 innermost broadcast; both use a
  pair-duplicate + int32-view copy chain so every wide op runs at 2x.
- Per block of 128 dst nodes: PE matmuls accumulate S^T @ [w*z~ | w] into
  PSUM, giving sum_e w*z~*[dst==j] and the softmax denominators.
  Epilogue: out = relu((psum_feat - xr~*denom) / denom + bias') where
  bias' = s*b; the |att| scale is left folded in h1 and undone via the
  host-transformed W2 matrices (relu commutes with positive scales), and
  the final output is unscaled/unpermuted on the host.
- Between layers: each core computes xl2~ = h1 @ W2l' for its slab
  (W2l' absorbs layer-1 unscale + layer-2 scale/permutation), AllGather
  (bf16) replicates the xl2~ table; xr2~ stays local.
"""
import sys
sys.path.insert(0, '/opt/trn_rl_repo')
import numpy as np
from dataclasses import dataclass

import concourse.bass as bass
import concourse.bacc as bacc
import concourse.mybir as mybir
from concourse.tile import TileContext
from concourse.library_config import mlp
from concourse.bass_utils import run_bass_kernel_spmd

P = 128
H, C = 4, 32
D = H * C          # 128
SLOPE = 0.2
F32 = mybir.dt.float32
BF16 = mybir.dt.bfloat16
I16 = mybir.dt.int16
I32 = mybir.dt.int32
NPBF = mybir.dt.np(BF16)


@dataclass
class Plan:
    N: int
    NC: int
    NPC: int        # nodes per core
    NBLK: int       # blocks per core
    SLAB: int       # NBLK*128
    G_lo: int
    G_hi: int
    split_rank: int

    @property
    def GPB(self):
        return self.G_lo + self.G_hi


def wrap_idx(flat):
    """[n] int -> dma_gather SBUF layout [128, n//16] (16-wrapped, 8x replicated)."""
    n = flat.shape[0]
    assert n % 16 == 0
    w = flat.reshape(n // 16, 16).T      # [16, n/16]
    return np.tile(w, (8, 1)).astype(np.int16)


def preprocess(x, edge_index, NC=8):
    """Build the per-core streams. Returns (plan, per_core_dict_list)."""
    N = x.shape[0]
    assert N % NC == 0
    NPC = N // NC
    NBLK = (NPC + P - 1) // P
    SLAB = NBLK * P
    split_rank = NC // 2
    SPLIT1 = split_rank * NPC          # layer-1 lo/hi split (global node id)
    assert SPLIT1 <= 32768 and N - SPLIT1 <= 32768
    assert split_rank * SLAB <= 32768 and (NC - split_rank) * SLAB <= 32768

    loop = np.arange(N, dtype=np.int64)
    src = np.concatenate([np.asarray(edge_index[0]), loop]).astype(np.int64)
    dst = np.concatenate([np.asarray(edge_index[1]), loop]).astype(np.int64)

    order = np.argsort(dst, kind='stable')
    src = src[order].astype(np.int32)
    dst = dst[order].astype(np.int32)

    core_bounds = np.searchsorted(dst, np.arange(NC + 1) * NPC)

    per_core = []
    G_lo = G_hi = 1
    for k in range(NC):
        a, b = core_bounds[k], core_bounds[k + 1]
        s_k = src[a:b]
        d_k = dst[a:b] - k * NPC
        blk = d_k // P
        is_lo = s_k < SPLIT1
        lo_counts = np.bincount(blk[is_lo], minlength=NBLK)
        hi_counts = np.bincount(blk[~is_lo], minlength=NBLK)
        G_lo = max(G_lo, int(np.max((lo_counts + P - 1) // P)) or 1)
        G_hi = max(G_hi, int(np.max((hi_counts + P - 1) // P)) or 1)
        per_core.append((s_k, d_k, blk, is_lo))

    plan = Plan(N=N, NC=NC, NPC=NPC, NBLK=NBLK, SLAB=SLAB,
                G_lo=G_lo, G_hi=G_hi, split_rank=split_rank)
    GPB = plan.GPB

    datas = []
    for k in range(NC):
        s_k, d_k, blk, is_lo = per_core[k]
        idxA1 = np.zeros((NBLK, GPB * P), np.int16)
        idxA2 = np.zeros((NBLK, GPB * P), np.int16)
        idxB = np.zeros((NBLK, GPB * P), np.int16)
        dstrel = np.full((NBLK, GPB * P), -1.0, np.float32)
        for b in range(NBLK):
            in_b = blk == b
            for side, G0, Gn in ((True, 0, G_lo), (False, G_lo, G_hi)):
                sel = in_b & (is_lo == side)
                ss = s_k[sel]
                dd = d_k[sel]
                n = ss.shape[0]
                assert n <= Gn * P
                o = G0 * P
                if side:
                    idxA1[b, o:o + n] = ss
                    idxA2[b, o:o + n] = (ss // NPC) * SLAB + (ss % NPC)
                else:
                    idxA1[b, o:o + n] = ss - SPLIT1
                    idxA2[b, o:o + n] = ((ss // NPC) * SLAB + (ss % NPC)
                                         - split_rank * SLAB)
                idxB[b, o:o + n] = dd
                dstrel[b, o:o + n] = dd - b * P

        def wrap_blocks(arr):
            return np.stack([wrap_idx(arr[b]) for b in range(NBLK)])

        wA1 = wrap_blocks(idxA1)
        wA2 = wrap_blocks(idxA2)
        wB = wrap_blocks(idxB)
        blkidx_l1 = np.concatenate([wA1, wB], axis=2).reshape(NBLK * P, 2 * GPB * 8)
        blkidx_l2 = np.concatenate([wA2, wB], axis=2).reshape(NBLK * P, 2 * GPB * 8)
        dr = dstrel.reshape(NBLK, GPB, P).transpose(0, 2, 1).reshape(NBLK * P, GPB)
        datas.append(dict(blkidx_l1=blkidx_l1, blkidx_l2=blkidx_l2,
                          dstrel=np.ascontiguousarray(dr).astype(NPBF)))
    return plan, datas


def sign_perm(att):
    """Permutation putting each head's positive-att columns first.

    Returns (perm[128], scales s=|att|[perm], pos-counts per head)."""
    a = np.asarray(att, np.float32).reshape(H, C)
    perm = []
    pcounts = []
    for h in range(H):
        pos = np.nonzero(a[h] >= 0)[0]
        neg = np.nonzero(a[h] < 0)[0]
        perm.extend((h * C + pos).tolist() + (h * C + neg).tolist())
        pcounts.append(len(pos))
    perm = np.asarray(perm, np.int64)
    flat = np.abs(a.reshape(-1))[perm]
    return perm, flat.astype(np.float32), pcounts


def build_kernel(plan, pcounts1, pcounts2, repeat=1):
    """Build the SPMD nc program (identical for all cores)."""
    pl = plan
    GPB, G_lo, G_hi, NBLK, SLAB = pl.GPB, pl.G_lo, pl.G_hi, pl.NBLK, pl.SLAB
    NLO1 = pl.split_rank * pl.NPC
    NLO2 = pl.split_rank * SLAB
    A = mybir.AluOpType

    NQ = 4  # SWDGE queues: round-robin the gathers so rings drain in parallel
    nc = bacc.Bacc("TRN2", target_bir_lowering=False, debug=False,
                   num_swdge_queues=NQ)
    qctr = [0]
    dp = lambda name, shape, dt=BF16, out=False: nc.declare_dram_parameter(
        name, list(shape), dt, isOutput=out).ap()

    xl1 = dp("xl1", [pl.N, D])
    xr1_loc = dp("xr1_loc", [SLAB, D])
    blkidx_l1 = dp("blkidx_l1", [NBLK * P, 2 * GPB * 8], I16)
    blkidx_l2 = dp("blkidx_l2", [NBLK * P, 2 * GPB * 8], I16)
    dstrel_p = dp("dstrel", [NBLK * P, GPB])
    iota_p = dp("iota", [P, P])
    ident_p = dp("ident", [P, P])
    W2l_p = dp("W2l", [D, D])
    W2r_p = dp("W2r", [D, D])
    bias1_p = dp("bias1", [P, D], F32)
    bias2_p = dp("bias2", [P, D], F32)
    out_p = dp("out", [SLAB, D], out=True)

    h1_loc = nc.dram_tensor("h1_loc", [SLAB, D], BF16).ap()
    xl2_slab = nc.dram_tensor("xl2_slab", [SLAB, D], BF16).ap()
    xl2_full = nc.dram_tensor("xl2_full", [pl.NC * SLAB, D], BF16,
                              addr_space="Shared").ap()
    xr2_loc = nc.dram_tensor("xr2_loc", [SLAB, D], BF16).ap()

    with TileContext(nc) as tc:
        nc.gpsimd.load_library(mlp)
        with (
            tc.tile_pool(name="const", bufs=1) as cpool,
            tc.tile_pool(name="stream", bufs=3) as spool,
            tc.tile_pool(name="work", bufs=2) as wpool,
            tc.tile_pool(name="small", bufs=3) as smpool,
            tc.tile_pool(name="psum", bufs=2, space="PSUM") as pspool,
            tc.tile_pool(name="psum2", bufs=2, space="PSUM") as ps2pool,
        ):
            iota_c = cpool.tile([P, P], BF16)
            nc.sync.dma_start(out=iota_c[:], in_=iota_p[:, :])
            ident_c = cpool.tile([P, P], BF16)
            nc.sync.dma_start(out=ident_c[:], in_=ident_p[:, :])
            W2l_c = cpool.tile([D, D], BF16)
            nc.sync.dma_start(out=W2l_c[:], in_=W2l_p[:, :])
            W2r_c = cpool.tile([D, D], BF16)
            nc.sync.dma_start(out=W2r_c[:], in_=W2r_p[:, :])
            bias1_c = cpool.tile([P, D], F32)
            nc.sync.dma_start(out=bias1_c[:], in_=bias1_p[:, :])
            bias2_c = cpool.tile([P, D], F32)
            nc.sync.dma_start(out=bias2_c[:], in_=bias2_p[:, :])
            alpha_c = cpool.tile([P, 1], F32)
            nc.vector.memset(alpha_c[:], SLOPE)

            def edge_layer(tab_lo, tab_hi, tab_B, blkidx, pcounts, bias_c,
                           out_rows, xr_loc_ap):
                sides = [(0, G_lo, tab_lo), (G_lo, G_hi, tab_hi)]
                sides = [s for s in sides if s[1] > 0]
                for b in range(NBLK):
                    idx_t = spool.tile([P, 2 * GPB * 8], I16, tag="idx")
                    nc.sync.dma_start(out=idx_t[:],
                                      in_=blkidx[b * P:(b + 1) * P, :])
                    dr_t = spool.tile([P, GPB], BF16, tag="dr")
                    nc.sync.dma_start(out=dr_t[:],
                                      in_=dstrel_p[b * P:(b + 1) * P, :])

                    za = wpool.tile([P, GPB, D], BF16, tag="za")
                    zb = wpool.tile([P, GPB, D], BF16, tag="zb")
                    GCH = 8  # ring limit: <=1024 idx (64 descs/lane) per call
                    for G0, Gn, tab in sides:
                        for g0 in range(0, Gn, GCH):
                            gn = min(GCH, Gn - g0)
                            nc.gpsimd.dma_gather(
                                out_ap=za[:, G0 + g0:G0 + g0 + gn, :], in_ap=tab,
                                idxs_ap=idx_t[:, (G0 + g0) * 8:(G0 + g0 + gn) * 8],
                                num_idxs=gn * P, num_idxs_reg=gn * P, elem_size=D,
                                queue_num=qctr[0] % NQ)
                            qctr[0] += 1
                            nc.gpsimd.dma_gather(
                                out_ap=zb[:, G0 + g0:G0 + g0 + gn, :], in_ap=tab_B,
                                idxs_ap=idx_t[:, (GPB + G0 + g0) * 8:
                                              (GPB + G0 + g0 + gn) * 8],
                                num_idxs=gn * P, num_idxs_reg=gn * P, elem_size=D,
                                queue_num=qctr[0] % NQ)
                            qctr[0] += 1

                    # z~ = xl~ + xr~ (in-place into za); Prelu -> zb
                    nc.vector.tensor_tensor(out=za[:], in0=za[:], in1=zb[:],
                                            op=A.add)
                    nc.scalar.activation(out=zb[:], in_=za[:],
                                         func=mybir.ActivationFunctionType.Prelu,
                                         alpha=alpha_c[:, :])
                    # sign fix: negate each head's negative-att column block
                    zbh = zb[:].rearrange("p g (h c) -> p g h c", h=H)
                    for h in range(H):
                        ph = pcounts[h]
                        if ph < C:
                            nc.vector.tensor_scalar_mul(
                                out=zbh[:, :, h, ph:C], in0=zbh[:, :, h, ph:C],
                                scalar1=-1.0)
                    # tree-reduce over c (2x-mode adds; last step to fp32)
                    e16 = smpool.tile([P, GPB, H, 16], BF16, tag="e16")
                    nc.vector.tensor_tensor(out=e16[:], in0=zbh[:, :, :, 0:16],
                                            in1=zbh[:, :, :, 16:32], op=A.add)
                    e8 = smpool.tile([P, GPB, H, 8], BF16, tag="e8")
                    nc.vector.tensor_tensor(out=e8[:], in0=e16[:, :, :, 0:8],
                                            in1=e16[:, :, :, 8:16], op=A.add)
                    e4 = smpool.tile([P, GPB, H, 4], BF16, tag="e4")
                    nc.vector.tensor_tensor(out=e4[:], in0=e8[:, :, :, 0:4],
                                            in1=e8[:, :, :, 4:8], op=A.add)
                    e2 = smpool.tile([P, GPB, H, 2], BF16, tag="e2")
                    nc.vector.tensor_tensor(out=e2[:], in0=e4[:, :, :, 0:2],
                                            in1=e4[:, :, :, 2:4], op=A.add)
                    e1 = smpool.tile([P, GPB, H], F32, tag="e1")
                    nc.vector.tensor_tensor(out=e1[:], in0=e2[:, :, :, 0],
                                            in1=e2[:, :, :, 1], op=A.add)
                    # w = exp(e)
                    w_t = smpool.tile([P, GPB, H], BF16, tag="w")
                    nc.scalar.activation(out=w_t[:], in_=e1[:],
                                         func=mybir.ActivationFunctionType.Exp)
                    # expand w to [P,G,H,C] via pair-dup + int32-view copies
                    wdup = smpool.tile([P, GPB, H, 2], BF16, tag="wdup")
                    nc.vector.tensor_copy(
                        out=wdup[:],
                        in_=w_t[:].unsqueeze(3).to_broadcast([P, GPB, H, 2]))
                    wfull = wpool.tile([P, GPB, D], BF16, tag="wfull")
                    nc.vector.tensor_copy(
                        out=wfull[:].bitcast(I32).rearrange(
                            "p g (h c) -> p g h c", h=H),
                        in_=wdup[:].bitcast(I32).to_broadcast(
                            [P, GPB, H, C // 2]))
                    # rhs = [w * z~ | w]
                    rhs = wpool.tile([P, GPB, D + H], BF16, tag="rhs")
                    nc.vector.tensor_tensor(out=rhs[:, :, 0:D], in0=za[:],
                                            in1=wfull[:], op=A.mult)
                    nc.vector.tensor_copy(out=rhs[:, :, D:D + H], in_=w_t[:])
                    # S = (iota == dstrel) via pair-dup + int32-view expand
                    drdup = smpool.tile([P, GPB, 2], BF16, tag="drdup")
                    nc.vector.tensor_copy(
                        out=drdup[:],
                        in_=dr_t[:].unsqueeze(2).to_broadcast([P, GPB, 2]))
                    drfull = wpool.tile([P, GPB, P], BF16, tag="drfull")
                    nc.vector.tensor_copy(
                        out=drfull[:].bitcast(I32),
                        in_=drdup[:].bitcast(I32).to_broadcast(
                            [P, GPB, P // 2]))
                    S_t = wpool.tile([P, GPB, P], BF16, tag="S")
                    nc.vector.tensor_tensor(
                        out=S_t[:],
                        in0=iota_c[:].unsqueeze(1).to_broadcast([P, GPB, P]),
                        in1=drfull[:], op=A.is_equal)

                    ps = pspool.tile([P, D + H], F32, tag="agg")
                    for gi in range(GPB):
                        nc.tensor.matmul(
                            out=ps[:], lhsT=S_t[:, gi, :], rhs=rhs[:, gi, :],
                            start=(gi == 0), stop=(gi == GPB - 1))

                    xrb = smpool.tile([P, D], BF16, tag="xrb")
                    nc.sync.dma_start(out=xrb[:],
                                      in_=xr_loc_ap[b * P:(b + 1) * P, :])
                    # denom > 0 always: every node has a self-loop edge
                    dinv = smpool.tile([P, H], F32, tag="dinv")
                    nc.vector.reciprocal(out=dinv[:], in_=ps[:, D:D + H])
                    t1 = smpool.tile([P, D], F32, tag="t1")
                    nc.vector.tensor_tensor(
                        out=t1[:].rearrange("p (h c) -> p h c", h=H),
                        in0=xrb[:].rearrange("p (h c) -> p h c", h=H),
                        in1=ps[:, D:D + H].unsqueeze(2).to_broadcast([P, H, C]),
                        op=A.mult)
                    t2 = smpool.tile([P, D], F32, tag="t2")
                    nc.vector.tensor_tensor(out=t2[:], in0=ps[:, 0:D], in1=t1[:],
                                            op=A.subtract)
                    t3 = smpool.tile([P, D], F32, tag="t3")
                    nc.vector.tensor_tensor(
                        out=t3[:].rearrange("p (h c) -> p h c", h=H),
                        in0=t2[:].rearrange("p (h c) -> p h c", h=H),
                        in1=dinv[:].unsqueeze(2).to_broadcast([P, H, C]),
                        op=A.mult)
                    t4 = smpool.tile([P, D], F32, tag="t4")
                    nc.vector.tensor_tensor(out=t4[:], in0=t3[:], in1=bias_c[:],
                                            op=A.add)
                    hrow = smpool.tile([P, D], BF16, tag="hrow")
                    nc.vector.tensor_scalar_max(out=hrow[:], in0=t4[:],
                                                scalar1=0.0)
                    nc.sync.dma_start(out=out_rows[b * P:(b + 1) * P, :],
                                      in_=hrow[:])

            for _rep in range(repeat):
                edge_layer(xl1[0:NLO1, :], xl1[NLO1:pl.N, :], xr1_loc[:, :],
                           blkidx_l1, pcounts1, bias1_c, h1_loc, xr1_loc)

                for b in range(NBLK):
                    htile = smpool.tile([P, D], BF16, tag="pl_h")
                    nc.sync.dma_start(out=htile[:],
                                      in_=h1_loc[b * P:(b + 1) * P, :])
                    psT = ps2pool.tile([P, P], BF16, tag="pl_T")
                    nc.tensor.transpose(out=psT[:], in_=htile[:],
                                        identity=ident_c[:])
                    hT = smpool.tile([P, P], BF16, tag="pl_hT")
                    nc.vector.tensor_copy(out=hT[:], in_=psT[:])
                    for W_c, table in ((W2l_c, xl2_slab), (W2r_c, xr2_loc)):
                        psm = ps2pool.tile([P, D], F32, tag="pl_mm")
                        nc.tensor.matmul(out=psm[:], lhsT=hT[:], rhs=W_c[:],
                                         start=True, stop=True)
                        res = smpool.tile([P, D], BF16, tag="pl_res")
                        nc.vector.tensor_copy(out=res[:], in_=psm[:])
                        nc.sync.dma_start(out=table[b * P:(b + 1) * P, :],
                                          in_=res[:])

                nc.gpsimd.collective_compute(
                    "AllGather", A.bypass,
                    replica_groups=[list(range(pl.NC))],
                    ins=[xl2_slab[:, :].opt()],
                    outs=[xl2_full[:, :].opt()],
                )

                edge_layer(xl2_full[0:NLO2, :], xl2_full[NLO2:pl.NC * SLAB, :],
                           xr2_loc[:, :], blkidx_l2, pcounts2, bias2_c, out_p,
                           xr2_loc)

    return nc


def make_host_tables(x, W1_l, W1_r, att1, b1, W2_l, W2_r, att2, b2):
    """Host-side scaled/permuted tables and constants."""
    perm1, s1, pc1 = sign_perm(att1)
    perm2, s2, pc2 = sign_perm(att2)
    x = np.asarray(x, np.float32)
    xl1 = (x @ np.asarray(W1_l, np.float32))[:, perm1] * s1
    xr1 = (x @ np.asarray(W1_r, np.float32))[:, perm1] * s1
    # W2': rows in perm1 order, unscaled by 1/s1; cols in perm2 order, scaled
    W2l = (np.asarray(W2_l, np.float32)[perm1][:, perm2] * s2) / s1[:, None]
    W2r = (np.asarray(W2_r, np.float32)[perm1][:, perm2] * s2) / s1[:, None]
    bias1 = np.asarray(b1, np.float32)[perm1] * s1
    bias2 = np.asarray(b2, np.float32)[perm2] * s2
    return dict(perm1=perm1, s1=s1, pc1=pc1, perm2=perm2, s2=s2, pc2=pc2,
                xl1=xl1.astype(NPBF), xr1=xr1.astype(NPBF),
                W2l=W2l.astype(NPBF), W2r=W2r.astype(NPBF),
                bias1=np.tile(bias1[None, :], (P, 1)).astype(np.float32),
                bias2=np.tile(bias2[None, :], (P, 1)).astype(np.float32))


def make_inputs(plan, datas, ht):
    pl = plan
    iota = np.tile(np.arange(P, dtype=np.float32)[None, :], (P, 1)).astype(NPBF)
    ident = np.eye(P, dtype=np.float32).astype(NPBF)

    in_maps = []
    for k in range(pl.NC):
        xr1_loc = np.zeros((pl.SLAB, D), NPBF)
        nreal = min(pl.NPC, pl.N - k * pl.NPC)
        xr1_loc[:nreal] = ht['xr1'][k * pl.NPC: k * pl.NPC + nreal]
        in_maps.append(dict(
            xl1=ht['xl1'],
            xr1_loc=xr1_loc,
            blkidx_l1=datas[k]["blkidx_l1"],
            blkidx_l2=datas[k]["blkidx_l2"],
            dstrel=datas[k]["dstrel"],
            iota=iota, ident=ident,
            W2l=ht['W2l'], W2r=ht['W2r'],
            bias1=ht['bias1'], bias2=ht['bias2'],
        ))
    return in_maps


def assemble_output(plan, results, ht):
    out = np.zeros((plan.N, D), np.float32)
    for k in range(plan.NC):
        out[k * plan.NPC:(k + 1) * plan.NPC] = \
            results[k]["out"][:plan.NPC].astype(np.float32)
    # undo layer-2 column scale+permutation
    full = np.empty_like(out)
    full[:, ht['perm2']] = out / ht['s2'][None, :]
    return full


def kernel(x, edge_index, W1_l, W1_r, att1, b1, W2_l, W2_r, att2, b2):
    x = np.ascontiguousarray(np.asarray(x, np.float32))
    edge_index = np.asarray(edge_index)
    plan, datas = preprocess(x, edge_index, NC=8)
    ht = make_host_tables(x, W1_l, W1_r, att1, b1, W2_l, W2_r, att2, b2)
    nc = build_kernel(plan, ht['pc1'], ht['pc2'])
    nc.compile()
    in_maps = make_inputs(plan, datas, ht)
    res = run_bass_kernel_spmd(nc, in_maps, core_ids=list(range(8)))
    return assemble_output(plan, res.results, ht)
